# revision 20
# baseline (speedup 1.0000x reference)
"""Trainium2 Bass kernel for nn_DeStationaryCausalAttention.

The reference returns only the LAST query position's output, so the full
L x L attention collapses: per batch we only need

    logits[h, k] = q_eff[h] . K[k, h-slice]      (k over all 2048 keys)
    out          = softmax(logits) @ V  -> @ Wo + bo

with q_eff = tau * q_c / sqrt(32) + delta_last.  Folding q_eff through Wk
gives a per-batch matrix G (16 x 1024) with logits = G @ h^T, and folding
Wv out of the weighted sum gives the output from u = softmax(logits) @ h.
The device computes logits = G @ h^T and the softmax partials (s, u) over
its shard of keys; the tiny rank-1 algebra (tau/delta MLPs on the last
row, G prep, output projection) is host math.

Sharding: the 4096 (batch, key) rows split into 8 chunks of 512 keys, one
per NeuronCore.  Per core the device reads h once in each layout it needs
as fp8 e3m4 (4-bit mantissa; measured end-to-end rel err ~8e-3, within
the 2e-2 gate), with G kept fp16 (G's small magnitudes quantize poorly):
 - h shard transposed (D-major) fp8 + G fp16, one DMA
 - h shard natural (key-major) fp8, four DMAs (the last key tile split at
   the 512B-descriptor boundary so only four matmuls trail its
   completion semaphore)
Logits stay < 3 in magnitude so exp needs no max subtraction.  u and the
softmax normalizer s accumulate across key tiles in a single PSUM
accumulation group (per-element first-touch zeroing lets one group span
all 36 matmuls), so no vector-engine adds are needed.  The output DMA's
descriptor generation is overlapped with the PSUM-evacuation copy (see
_overlap_out_dma), hiding most of the HWDGE + DGE latency in the tail.
"""

import math

import numpy as np

# Problem shapes (hardcoded per the harness contract).
B, L, D = 2, 2048, 1024
H, HD, KVHD, DKV = 16, 64, 32, 512
NCORES = 8
CHUNK = (B * L) // NCORES       # 512 keys per core
P = 128
KT = CHUNK // P                 # 4 key tiles per core
DT = D // P                     # 8 model-dim tiles

HTF_B = KT * DT * P             # 4096 bytes of transposed h per partition
G_B = DT * H * 2                # 256 bytes of fp16 G per partition
A_B = HTF_B + G_B               # first-DMA row bytes
OUT_F = P + 1                   # output row: 128 u columns + the s column
D1 = 4 * P                      # last hnf tile's split point (512B halves)

_CACHE = {}


def _fix_sync_waits(nc, maxw=1):
    """Walrus (CoreV3) rejects instructions carrying more than one sync-wait
    command.  Tile's end-of-kernel drain collects one wait per outstanding
    semaphore, so split excess waits onto preceding same-engine NoOps."""
    import concourse.mybir as mybir

    engines = [mybir.EngineType.SP, mybir.EngineType.DVE,
               mybir.EngineType.Activation, mybir.EngineType.PE,
               mybir.EngineType.Pool]
    ctr = 0
    first_block = True
    for fn in nc.m.functions:
        for blk in fn.blocks:
            if first_block:
                # Drop the preamble's drain + all-engine EVSEM barrier.
                # Engines only initialize their own registers, semaphores are
                # cleared by the previous execution's tail, and the only
                # cross-engine preamble product (Pool's const-tile memsets,
                # done <1us) is first read by ACT's exp well after 1us.
                first_block = False
                insts = blk.instructions
                head_end = next(
                    (i for i, ins in enumerate(insts)
                     if type(ins).__name__ == "InstUnconditionalBranch"),
                    0)
                pruned = [ins for i, ins in enumerate(insts)
                          if not (i < head_end and type(ins).__name__ in
                                  ("InstDrain", "InstEventSemaphore"))]
                if len(pruned) != len(insts):
                    blk.instructions = pruned
            new = []
            changed = False
            for inst in blk.instructions:
                si = inst.sync_info
                if si is not None and si.on_wait and len(si.on_wait) > maxw:
                    waits = list(si.on_wait)
                    extra, keep = waits[:-maxw], waits[-maxw:]
                    spread = type(inst).__name__ == "InstDrain"
                    for i in range(0, len(extra), maxw):
                        nop = mybir.InstNoOp(
                            name=f"waitfix-{ctr}", ins=[], outs=[])
                        nop.engine = (engines[ctr % len(engines)]
                                      if spread else inst.engine)
                        ctr += 1
                        nop.sync_info = mybir.SyncInfo(
                            on_wait=extra[i:i + maxw], on_update=[])
                        new.append(nop)
                    si.on_wait = keep
                    changed = True
                new.append(inst)
            if changed:
                blk.instructions = new


def _trim_tail_barrier(nc):
    """Drop the second end-of-kernel all-engine barrier.  It only holds the
    other engines alive until Pool's semaphore-clear ISA op finishes, but
    NEFF completion already requires Pool's own halt, which follows the
    clear; the clear itself stays ordered after barrier 1."""
    blk = nc.m.functions[0].blocks[-1]
    insts = blk.instructions
    isa_idx = max((i for i, ins in enumerate(insts)
                   if type(ins).__name__ == "InstISA"), default=None)
    if isa_idx is not None and isa_idx + 1 < len(insts):
        tail = insts[isa_idx + 1:]
        if all(type(t).__name__ in ("InstDrain", "InstEventSemaphore")
               for t in tail):
            blk.instructions = insts[:isa_idx + 1]


def _build_nc():
    from contextlib import ExitStack

    import concourse.bass as bass
    import concourse.tile as tile
    from concourse import mybir

    f32 = mybir.dt.float32  # noqa: F841 — PSUM accumulators only
    f16 = mybir.dt.float16
    f8 = mybir.dt.float8e3
    u8 = mybir.dt.uint8
    nc = bass.Bass("TRN2", debug=False, num_devices=NCORES)

    hA_d = nc.dram_tensor("hA", [P, A_B], u8, kind="ExternalInput").ap()
    hB_d = nc.dram_tensor("hB", [P, 2 * D], f8, kind="ExternalInput").ap()
    hC_d = nc.dram_tensor("hC", [P, D], f8, kind="ExternalInput").ap()
    hD_d = nc.dram_tensor("hD", [P, D], f8, kind="ExternalInput").ap()
    out_d = nc.dram_tensor("ut_out", [P, OUT_F], f16, kind="ExternalOutput").ap()

    with tile.TileContext(nc) as tc, ExitStack() as ctx:
        consts = ctx.enter_context(tc.tile_pool(name="consts", bufs=1))
        hp = ctx.enter_context(tc.tile_pool(name="hp", bufs=1))
        small = ctx.enter_context(tc.tile_pool(name="small", bufs=1))
        pslg = ctx.enter_context(tc.tile_pool(name="pslg", bufs=4, space="PSUM"))
        psu = ctx.enter_context(tc.tile_pool(name="psu", bufs=1, space="PSUM"))

        # ---- input DMAs: transposed h + G + idxs first, natural h after ----
        tA = hp.tile([P, A_B], u8, tag="hA")
        nc.sync.dma_start(tA[:], hA_d[:])
        tB = hp.tile([P, 2, D], f8, tag="hB")
        nc.sync.dma_start(tB[:], hB_d[:].rearrange("p (a c) -> p a c", a=2))
        tC = hp.tile([P, D], f8, tag="hC")
        nc.sync.dma_start(tC[:], hC_d[:])
        # last natural tile split so only two matmuls trail the final
        # DMA-completion semaphore
        tD1 = hp.tile([P, D1], f8, tag="hD1")
        nc.sync.dma_start(tD1[:], hD_d[:, 0:D1])
        tD2 = hp.tile([P, D - D1], f8, tag="hD2")
        nc.sync.dma_start(tD2[:], hD_d[:, D1:D])

        g16 = tA[:, HTF_B:HTF_B + G_B].bitcast(f16)          # [128, 128]

        ones_sb = consts.tile([P, 1], f16)
        nc.vector.memset(ones_sb[:], 1.0)
        outsb = small.tile([P, OUT_F], f16, tag="outsb")

        # one tile per key tile so each u-matmul group depends only on its
        # own exp, not on later writes into a shared buffer
        pts = [small.tile([P, H], f16, tag=f"pt{kt}", name=f"pt{kt}")
               for kt in range(KT)]

        # ---- logits + exp, all fed by DMA A ----
        for kt in range(KT):
            ps_lg = pslg.tile([P, H], f32, tag="lg")
            for dt in range(DT):
                w = tA[:, (kt * DT + dt) * P:(kt * DT + dt + 1) * P].bitcast(f8)
                nc.tensor.matmul(
                    ps_lg[:], w, g16[:, dt * H:(dt + 1) * H],
                    start=(dt == 0), stop=(dt == DT - 1))
            nc.scalar.activation(
                pts[kt][:], ps_lg[:], mybir.ActivationFunctionType.Exp,
                bias=0.0, scale=1.0)

        # ---- u and s in one cross-kt PSUM accumulation group --------------
        # u[:, dt*16+h] += hnf_kt[:, dt-block]^T p_kt ; s[h, 0] += 1^T p_kt.
        # First-touch zeroing inside the group's zero region makes kt 0 a
        # write and kt 1..3 accumulations, so no start/stop per tile.
        ps_u = psu.tile([P, 132], f32, tag="u_acc")

        def u_block(kt, dts, src):
            for dt in dts:
                nc.tensor.matmul(
                    ps_u[:, dt * H:(dt + 1) * H],
                    src[:, (dt - dts[0]) * P:(dt - dts[0] + 1) * P],
                    pts[kt][:],
                    start=(kt == 0 and dt == 0), stop=False,
                    skip_group_check=True)

        for kt, src in ((0, tB[:, 0, :]), (1, tB[:, 1, :]), (2, tC[:])):
            u_block(kt, range(DT), src)
            nc.tensor.matmul(
                ps_u[0:H, 128:129], pts[kt][:], ones_sb[:],
                start=False, stop=False, skip_group_check=True)
        u_block(3, range(4), tD1[:])
        u_block(3, range(4, DT), tD2[:])
        nc.tensor.matmul(
            ps_u[0:H, 128:129], pts[3][:], ones_sb[:],
            start=False, stop=True, skip_group_check=True)

        # ---- evacuate PSUM and ship the result ----------------------------
        # one copy spanning u plus the s column; the s column's partitions
        # 16..127 are never written and carry garbage the host ignores
        nc.vector.tensor_copy(outsb[:], ps_u[:, 0:OUT_F])
        nc.sync.dma_start(out_d[:], outsb[:])

    _overlap_out_dma(nc)
    _fix_sync_waits(nc)
    _trim_tail_barrier(nc)
    return nc


def _overlap_out_dma(nc):
    """Start the output DMA's descriptor generation under the PSUM-evacuation
    copy instead of after it.  The HWDGE gen + DGE-to-SDMA delay put >=1.2us
    between the doorbell and the SBUF read, while the DVE copy retires
    ~0.4us after the same gating event (the accumulation group's stop
    matmul), so the transfer still reads fully-written data with wide
    margin.  Swap the DMA's wait (on the DVE copy) for the copy's own wait
    (on the PE stop matmul)."""
    out_dma = None
    dve_copy = None
    for blk in nc.m.functions[0].blocks:
        for ins in blk.instructions:
            tn = type(ins).__name__
            if tn == "InstDMACopy" and ins.outs and "ut_out" in str(
                    getattr(ins.outs[0], "memref", "")):
                out_dma = ins
            if (tn == "InstTensorCopy"
                    and "outsb" in str(getattr(ins.outs[0], "memref", ""))):
                dve_copy = ins
    assert out_dma is not None and dve_copy is not None
    pe_waits = [w for w in dve_copy.sync_info.on_wait
                if w.ant_name and w.ant_name.startswith("PE")]
    assert pe_waits, [w.ant_name for w in dve_copy.sync_info.on_wait]
    out_dma.sync_info.on_wait = list(pe_waits)



def _get_nc():
    if "nc" not in _CACHE:
        _CACHE["nc"] = _build_nc()
    return _CACHE["nc"]


def _gelu_exact(x):
    # erf-based GELU, matches jax.nn.gelu(approximate=False).
    from math import erf
    v = np.vectorize(erf, otypes=[np.float64])
    return 0.5 * x * (1.0 + v(x / math.sqrt(2.0)))


def kernel(h, pre_norm_mu, pre_norm_sigma, Wq, Wk, Wv, Wo, bo,
           tau_w1, tau_b1, tau_w2, tau_b2, del_w1, del_b1, del_w2, del_b2):
    import ml_dtypes
    from concourse.bass_utils import run_bass_kernel_spmd

    e3 = ml_dtypes.float8_e3m4
    h = np.asarray(h, np.float32)
    f8 = np.float64

    # --- tiny host math for the last position -------------------------------
    h_last = h[:, -1, :].astype(f8)                                   # (B, D)
    sig_mean = np.clip(
        np.asarray(pre_norm_sigma, f8)[:, -1, :].mean(-1, keepdims=True),
        1e-6, None)
    mu_mean = np.asarray(pre_norm_mu, f8)[:, -1, :].mean(-1, keepdims=True)

    tau = np.exp(np.clip(
        _gelu_exact(np.concatenate([sig_mean, h_last], -1)
                    @ np.asarray(tau_w1, f8) + np.asarray(tau_b1, f8))
        @ np.asarray(tau_w2, f8) + np.asarray(tau_b2, f8), -3.0, 3.0))
    delta = np.clip(
        _gelu_exact(np.concatenate([mu_mean, h_last], -1)
                    @ np.asarray(del_w1, f8) + np.asarray(del_b1, f8))
        @ np.asarray(del_w2, f8) + np.asarray(del_b2, f8), -5.0, 5.0)

    q = h_last @ np.asarray(Wq, f8)                                   # (B, D)
    qc = q.reshape(B, H, HD)[:, :, :KVHD]                             # (B,H,32)
    q_eff = (tau.reshape(B, 1, 1) * qc / math.sqrt(KVHD)
             + delta.reshape(B, H, KVHD))
    Wk_r = np.asarray(Wk, f8).reshape(D, H, KVHD)
    G = np.einsum('bhd,Dhd->bhD', q_eff, Wk_r)                        # (B,H,D)
    # gt in the device SBUF layout: g16[p, dt*H + h] = G[h, dt*128 + p]
    Gt = np.ascontiguousarray(
        G.reshape(B, H, DT, P).transpose(0, 3, 2, 1)
    ).astype(np.float16).reshape(B, P, DT * H)
    G_bytes = Gt.view(np.uint8)                                       # (B,P,256)

    # --- device inputs ------------------------------------------------------
    in_maps = []
    for c in range(NCORES):
        b, ck = divmod(c, NCORES // B)
        hc = h[b, ck * CHUNK:(ck + 1) * CHUNK, :]                     # (512, D)
        h8 = hc.astype(e3)
        # htf bytes[p, (kt*8+dt)*128 + j] = h8[kt*128 + j, dt*128 + p]
        htf_b = np.ascontiguousarray(
            h8.view(np.uint8).reshape(KT, P, DT, P).transpose(3, 0, 2, 1)
        ).reshape(P, HTF_B)
        hA = np.concatenate([htf_b, G_bytes[b]], axis=1)
        in_maps.append({
            "hA": np.ascontiguousarray(hA),
            "hB": np.ascontiguousarray(
                np.concatenate([h8[0:P], h8[P:2 * P]], axis=1)),
            "hC": np.ascontiguousarray(h8[2 * P:3 * P]),
            "hD": np.ascontiguousarray(h8[3 * P:4 * P]),
        })
    _CACHE["last_in_maps"] = in_maps
    res = run_bass_kernel_spmd(_get_nc(), in_maps, core_ids=list(range(NCORES)))
    results = res.results

    # --- combine partials + output projection -------------------------------
    nshard = NCORES // B
    out = np.zeros((B, D), np.float32)
    Wv_r = np.asarray(Wv, f8).reshape(D, H, KVHD)
    for b in range(B):
        S = np.zeros(H, f8)
        U = np.zeros((H, D), f8)
        for ck in range(nshard):
            raw = results[b * nshard + ck]["ut_out"].astype(f8)
            S += raw[:H, 128]
            # ut_out[p, dt*H + h] = u[h, dt*128 + p]
            U += raw[:, :DT * H].reshape(P, DT, H).transpose(2, 1, 0).reshape(H, D)
        un = U / S[:, None]
        att = np.einsum('hD,Dhd->hd', un, Wv_r)                       # (H, 32)
        out[b] = (att.reshape(DKV) @ np.asarray(Wo, f8)
                  + np.asarray(bo, f8)).astype(np.float32)
    return out


# revision 22
# speedup vs baseline: 1.0217x; 1.0217x over previous
"""Trainium2 Bass kernel for nn_DeStationaryCausalAttention.

The reference returns only the LAST query position's output, so the full
L x L attention collapses: per batch we only need

    logits[h, k] = q_eff[h] . K[k, h-slice]      (k over all 2048 keys)
    out          = softmax(logits) @ V  -> @ Wo + bo

with q_eff = tau * q_c / sqrt(32) + delta_last.  Folding q_eff through Wk
gives a per-batch matrix G (16 x 1024) with logits = G @ h^T, and folding
Wv out of the weighted sum gives the output from u = softmax(logits) @ h.
The device computes logits = G @ h^T and the softmax partials (s, u) over
its shard of keys; the tiny rank-1 algebra (tau/delta MLPs on the last
row, G prep, output projection) is host math.

Sharding: the 4096 (batch, key) rows split into 8 chunks of 512 keys, one
per NeuronCore.  Per core the device reads h once in each layout it needs
as fp8 e3m4 (4-bit mantissa; measured end-to-end rel err ~8e-3, within
the 2e-2 gate), with G kept fp16 (G's small magnitudes quantize poorly):
 - h shard transposed (D-major) fp8 + G fp16, one DMA
 - h shard natural (key-major) fp8, four DMAs (the last key tile split at
   the 512B-descriptor boundary so only four matmuls trail its
   completion semaphore)
Logits stay < 3 in magnitude so exp needs no max subtraction.  u and the
softmax normalizer s accumulate across key tiles in a single PSUM
accumulation group (per-element first-touch zeroing lets one group span
all 36 matmuls), so no vector-engine adds are needed.  The output DMA's
descriptor generation is overlapped with the PSUM-evacuation copy (see
_overlap_out_dma), hiding most of the HWDGE + DGE latency in the tail.
"""

import math

import numpy as np

# Problem shapes (hardcoded per the harness contract).
B, L, D = 2, 2048, 1024
H, HD, KVHD, DKV = 16, 64, 32, 512
NCORES = 8
CHUNK = (B * L) // NCORES       # 512 keys per core
P = 128
KT = CHUNK // P                 # 4 key tiles per core
DT = D // P                     # 8 model-dim tiles

HTF_B = KT * DT * P             # 4096 bytes of transposed h per partition
HALF_B = HTF_B // 2             # transposed bytes per A-DMA half
G_B = DT * H * 2                # 256 bytes of fp16 G per partition
A1_B = HALF_B + G_B             # first DMA: htf kt0-1 + G
A2_B = HALF_B                   # second DMA: htf kt2-3
OUT_F = P + 1                   # output row: 128 u columns + the s column
D1 = 4 * P                      # last hnf tile's split point (512B halves)

_CACHE = {}


def _fix_sync_waits(nc, maxw=1):
    """Walrus (CoreV3) rejects instructions carrying more than one sync-wait
    command.  Tile's end-of-kernel drain collects one wait per outstanding
    semaphore, so split excess waits onto preceding same-engine NoOps."""
    import concourse.mybir as mybir

    engines = [mybir.EngineType.SP, mybir.EngineType.DVE,
               mybir.EngineType.Activation, mybir.EngineType.PE,
               mybir.EngineType.Pool]
    ctr = 0
    first_block = True
    for fn in nc.m.functions:
        for blk in fn.blocks:
            if first_block:
                # Drop the preamble's drain + all-engine EVSEM barrier.
                # Engines only initialize their own registers, semaphores are
                # cleared by the previous execution's tail, and the only
                # cross-engine preamble product (Pool's const-tile memsets,
                # done <1us) is first read by ACT's exp well after 1us.
                first_block = False
                insts = blk.instructions
                head_end = next(
                    (i for i, ins in enumerate(insts)
                     if type(ins).__name__ == "InstUnconditionalBranch"),
                    0)
                pruned = [ins for i, ins in enumerate(insts)
                          if not (i < head_end and type(ins).__name__ in
                                  ("InstDrain", "InstEventSemaphore"))]
                if len(pruned) != len(insts):
                    blk.instructions = pruned
            new = []
            changed = False
            for inst in blk.instructions:
                si = inst.sync_info
                if si is not None and si.on_wait and len(si.on_wait) > maxw:
                    waits = list(si.on_wait)
                    extra, keep = waits[:-maxw], waits[-maxw:]
                    spread = type(inst).__name__ == "InstDrain"
                    for i in range(0, len(extra), maxw):
                        nop = mybir.InstNoOp(
                            name=f"waitfix-{ctr}", ins=[], outs=[])
                        nop.engine = (engines[ctr % len(engines)]
                                      if spread else inst.engine)
                        ctr += 1
                        nop.sync_info = mybir.SyncInfo(
                            on_wait=extra[i:i + maxw], on_update=[])
                        new.append(nop)
                    si.on_wait = keep
                    changed = True
                new.append(inst)
            if changed:
                blk.instructions = new


def _trim_tail_barrier(nc):
    """Drop the second end-of-kernel all-engine barrier.  It only holds the
    other engines alive until Pool's semaphore-clear ISA op finishes, but
    NEFF completion already requires Pool's own halt, which follows the
    clear; the clear itself stays ordered after barrier 1."""
    blk = nc.m.functions[0].blocks[-1]
    insts = blk.instructions
    isa_idx = max((i for i, ins in enumerate(insts)
                   if type(ins).__name__ == "InstISA"), default=None)
    if isa_idx is not None and isa_idx + 1 < len(insts):
        tail = insts[isa_idx + 1:]
        if all(type(t).__name__ in ("InstDrain", "InstEventSemaphore")
               for t in tail):
            blk.instructions = insts[:isa_idx + 1]


def _build_nc():
    from contextlib import ExitStack

    import concourse.bass as bass
    import concourse.tile as tile
    from concourse import mybir

    f32 = mybir.dt.float32  # noqa: F841 — PSUM accumulators only
    f16 = mybir.dt.float16
    f8 = mybir.dt.float8e3
    u8 = mybir.dt.uint8
    nc = bass.Bass("TRN2", debug=False, num_devices=NCORES)

    hA1_d = nc.dram_tensor("hA1", [P, A1_B], u8, kind="ExternalInput").ap()
    hA2_d = nc.dram_tensor("hA2", [P, A2_B], u8, kind="ExternalInput").ap()
    hB_d = nc.dram_tensor("hB", [P, 2 * D], f8, kind="ExternalInput").ap()
    hC_d = nc.dram_tensor("hC", [P, D], f8, kind="ExternalInput").ap()
    hD_d = nc.dram_tensor("hD", [P, D], f8, kind="ExternalInput").ap()
    out_d = nc.dram_tensor("ut_out", [P, OUT_F], f16, kind="ExternalOutput").ap()

    with tile.TileContext(nc) as tc, ExitStack() as ctx:
        consts = ctx.enter_context(tc.tile_pool(name="consts", bufs=1))
        hp = ctx.enter_context(tc.tile_pool(name="hp", bufs=1))
        small = ctx.enter_context(tc.tile_pool(name="small", bufs=1))
        pslg = ctx.enter_context(tc.tile_pool(name="pslg", bufs=4, space="PSUM"))
        psu = ctx.enter_context(tc.tile_pool(name="psu", bufs=1, space="PSUM"))

        # ---- input DMAs: transposed h + G + idxs first, natural h after ----
        tA1 = hp.tile([P, A1_B], u8, tag="hA1")
        nc.sync.dma_start(tA1[:], hA1_d[:])
        tA2 = hp.tile([P, A2_B], u8, tag="hA2")
        nc.sync.dma_start(tA2[:], hA2_d[:])
        tB = hp.tile([P, 2, D], f8, tag="hB")
        nc.sync.dma_start(tB[:], hB_d[:].rearrange("p (a c) -> p a c", a=2))
        tC = hp.tile([P, D], f8, tag="hC")
        nc.sync.dma_start(tC[:], hC_d[:])
        # last natural tile split so only two matmuls trail the final
        # DMA-completion semaphore
        tD1 = hp.tile([P, D1], f8, tag="hD1")
        nc.sync.dma_start(tD1[:], hD_d[:, 0:D1])
        tD2 = hp.tile([P, D - D1], f8, tag="hD2")
        nc.sync.dma_start(tD2[:], hD_d[:, D1:D])

        g16 = tA1[:, HALF_B:HALF_B + G_B].bitcast(f16)       # [128, 128]

        ones_sb = consts.tile([P, 1], f16)
        nc.vector.memset(ones_sb[:], 1.0)
        outsb = small.tile([P, OUT_F], f16, tag="outsb")

        # one tile per key tile so each u-matmul group depends only on its
        # own exp, not on later writes into a shared buffer
        pts = [small.tile([P, H], f16, tag=f"pt{kt}", name=f"pt{kt}")
               for kt in range(KT)]

        # ---- logits + exp, all fed by DMA A ----
        for kt in range(KT):
            ps_lg = pslg.tile([P, H], f32, tag="lg")
            src_t, ko = (tA1, kt) if kt < 2 else (tA2, kt - 2)
            for dt in range(DT):
                w = src_t[:, (ko * DT + dt) * P:(ko * DT + dt + 1) * P].bitcast(f8)
                nc.tensor.matmul(
                    ps_lg[:], w, g16[:, dt * H:(dt + 1) * H],
                    start=(dt == 0), stop=(dt == DT - 1))
            nc.scalar.activation(
                pts[kt][:], ps_lg[:], mybir.ActivationFunctionType.Exp,
                bias=0.0, scale=1.0)

        # ---- u and s in one cross-kt PSUM accumulation group --------------
        # u[:, dt*16+h] += hnf_kt[:, dt-block]^T p_kt ; s[h, 0] += 1^T p_kt.
        # First-touch zeroing inside the group's zero region makes kt 0 a
        # write and kt 1..3 accumulations, so no start/stop per tile.
        ps_u = psu.tile([P, 132], f32, tag="u_acc")

        def u_block(kt, dts, src):
            for dt in dts:
                nc.tensor.matmul(
                    ps_u[:, dt * H:(dt + 1) * H],
                    src[:, (dt - dts[0]) * P:(dt - dts[0] + 1) * P],
                    pts[kt][:],
                    start=(kt == 0 and dt == 0), stop=False,
                    skip_group_check=True)

        for kt, src in ((0, tB[:, 0, :]), (1, tB[:, 1, :]), (2, tC[:])):
            u_block(kt, range(DT), src)
            nc.tensor.matmul(
                ps_u[0:H, 128:129], pts[kt][:], ones_sb[:],
                start=False, stop=False, skip_group_check=True)
        u_block(3, range(4), tD1[:])
        u_block(3, range(4, DT), tD2[:])
        nc.tensor.matmul(
            ps_u[0:H, 128:129], pts[3][:], ones_sb[:],
            start=False, stop=True, skip_group_check=True)

        # ---- evacuate PSUM and ship the result ----------------------------
        # one copy spanning u plus the s column; the s column's partitions
        # 16..127 are never written and carry garbage the host ignores
        nc.vector.tensor_copy(outsb[:], ps_u[:, 0:OUT_F])
        nc.sync.dma_start(out_d[:], outsb[:])

    _overlap_out_dma(nc)
    _early_last_ldweights(nc)
    _fix_sync_waits(nc)
    _trim_tail_barrier(nc)
    return nc


def _early_last_ldweights(nc):
    """The final u matmuls wait on the last input DMA's completion sem =
    its copy end + 900ns of sem propagation.  The DMA queue is in-order,
    so the second-to-last DMA's sem — firing 900ns after ITS copy, which
    is ~718ns after the last 182ns transfer already completed — equally
    guarantees the last tile's data is resident, holding even if the real
    propagation latency is several times shorter than modeled.  Retarget
    the one Ldweights wait."""
    d1_wait = None
    d2_waits = []
    for blk in nc.m.functions[0].blocks:
        for ins in blk.instructions:
            if type(ins).__name__ != "InstLdweights" or ins.sync_info is None:
                continue
            for w in ins.sync_info.on_wait:
                if w.ant_name and w.ant_name.startswith("DMAHW4"):
                    d1_wait = w
                elif w.ant_name and w.ant_name.startswith("DMAHW5"):
                    d2_waits.append(w)
    assert d1_wait is not None and d2_waits, (d1_wait, d2_waits)
    for w in d2_waits:
        w.id = d1_wait.id
        w.ant_name = d1_wait.ant_name
        w.wait_value = d1_wait.wait_value


def _overlap_out_dma(nc):
    """Start the output DMA's descriptor generation under the PSUM-evacuation
    copy instead of after it.  The HWDGE gen + DGE-to-SDMA delay put >=1.2us
    between the doorbell and the SBUF read, while the DVE copy retires
    ~0.4us after the same gating event (the accumulation group's stop
    matmul), so the transfer still reads fully-written data with wide
    margin.  Swap the DMA's wait (on the DVE copy) for the copy's own wait
    (on the PE stop matmul)."""
    out_dma = None
    dve_copy = None
    for blk in nc.m.functions[0].blocks:
        for ins in blk.instructions:
            tn = type(ins).__name__
            if tn == "InstDMACopy" and ins.outs and "ut_out" in str(
                    getattr(ins.outs[0], "memref", "")):
                out_dma = ins
            if (tn == "InstTensorCopy"
                    and "outsb" in str(getattr(ins.outs[0], "memref", ""))):
                dve_copy = ins
    assert out_dma is not None and dve_copy is not None
    pe_waits = [w for w in dve_copy.sync_info.on_wait
                if w.ant_name and w.ant_name.startswith("PE")]
    assert pe_waits, [w.ant_name for w in dve_copy.sync_info.on_wait]
    out_dma.sync_info.on_wait = list(pe_waits)



def _get_nc():
    if "nc" not in _CACHE:
        _CACHE["nc"] = _build_nc()
    return _CACHE["nc"]


def _gelu_exact(x):
    # erf-based GELU, matches jax.nn.gelu(approximate=False).
    from math import erf
    v = np.vectorize(erf, otypes=[np.float64])
    return 0.5 * x * (1.0 + v(x / math.sqrt(2.0)))


def kernel(h, pre_norm_mu, pre_norm_sigma, Wq, Wk, Wv, Wo, bo,
           tau_w1, tau_b1, tau_w2, tau_b2, del_w1, del_b1, del_w2, del_b2):
    import ml_dtypes
    from concourse.bass_utils import run_bass_kernel_spmd

    e3 = ml_dtypes.float8_e3m4
    h = np.asarray(h, np.float32)
    f8 = np.float64

    # --- tiny host math for the last position -------------------------------
    h_last = h[:, -1, :].astype(f8)                                   # (B, D)
    sig_mean = np.clip(
        np.asarray(pre_norm_sigma, f8)[:, -1, :].mean(-1, keepdims=True),
        1e-6, None)
    mu_mean = np.asarray(pre_norm_mu, f8)[:, -1, :].mean(-1, keepdims=True)

    tau = np.exp(np.clip(
        _gelu_exact(np.concatenate([sig_mean, h_last], -1)
                    @ np.asarray(tau_w1, f8) + np.asarray(tau_b1, f8))
        @ np.asarray(tau_w2, f8) + np.asarray(tau_b2, f8), -3.0, 3.0))
    delta = np.clip(
        _gelu_exact(np.concatenate([mu_mean, h_last], -1)
                    @ np.asarray(del_w1, f8) + np.asarray(del_b1, f8))
        @ np.asarray(del_w2, f8) + np.asarray(del_b2, f8), -5.0, 5.0)

    q = h_last @ np.asarray(Wq, f8)                                   # (B, D)
    qc = q.reshape(B, H, HD)[:, :, :KVHD]                             # (B,H,32)
    q_eff = (tau.reshape(B, 1, 1) * qc / math.sqrt(KVHD)
             + delta.reshape(B, H, KVHD))
    Wk_r = np.asarray(Wk, f8).reshape(D, H, KVHD)
    G = np.einsum('bhd,Dhd->bhD', q_eff, Wk_r)                        # (B,H,D)
    # gt in the device SBUF layout: g16[p, dt*H + h] = G[h, dt*128 + p]
    Gt = np.ascontiguousarray(
        G.reshape(B, H, DT, P).transpose(0, 3, 2, 1)
    ).astype(np.float16).reshape(B, P, DT * H)
    G_bytes = Gt.view(np.uint8)                                       # (B,P,256)

    # --- device inputs ------------------------------------------------------
    in_maps = []
    for c in range(NCORES):
        b, ck = divmod(c, NCORES // B)
        hc = h[b, ck * CHUNK:(ck + 1) * CHUNK, :]                     # (512, D)
        h8 = hc.astype(e3)
        # htf bytes[p, (kt*8+dt)*128 + j] = h8[kt*128 + j, dt*128 + p]
        htf_b = np.ascontiguousarray(
            h8.view(np.uint8).reshape(KT, P, DT, P).transpose(3, 0, 2, 1)
        ).reshape(P, HTF_B)
        in_maps.append({
            "hA1": np.ascontiguousarray(
                np.concatenate([htf_b[:, :HALF_B], G_bytes[b]], axis=1)),
            "hA2": np.ascontiguousarray(htf_b[:, HALF_B:]),
            "hB": np.ascontiguousarray(
                np.concatenate([h8[0:P], h8[P:2 * P]], axis=1)),
            "hC": np.ascontiguousarray(h8[2 * P:3 * P]),
            "hD": np.ascontiguousarray(h8[3 * P:4 * P]),
        })
    _CACHE["last_in_maps"] = in_maps
    res = run_bass_kernel_spmd(_get_nc(), in_maps, core_ids=list(range(NCORES)))
    results = res.results

    # --- combine partials + output projection -------------------------------
    nshard = NCORES // B
    out = np.zeros((B, D), np.float32)
    Wv_r = np.asarray(Wv, f8).reshape(D, H, KVHD)
    for b in range(B):
        S = np.zeros(H, f8)
        U = np.zeros((H, D), f8)
        for ck in range(nshard):
            raw = results[b * nshard + ck]["ut_out"].astype(f8)
            S += raw[:H, 128]
            # ut_out[p, dt*H + h] = u[h, dt*128 + p]
            U += raw[:, :DT * H].reshape(P, DT, H).transpose(2, 1, 0).reshape(H, D)
        un = U / S[:, None]
        att = np.einsum('hD,Dhd->hd', un, Wv_r)                       # (H, 32)
        out[b] = (att.reshape(DKV) @ np.asarray(Wo, f8)
                  + np.asarray(bo, f8)).astype(np.float32)
    return out
